# revision 5
# baseline (speedup 1.0000x reference)
"""Trainium2 raw-Bass CRF kernel, v5: v4 time-segmentation + second
elementwise path.

Same 38-slot R=4 segmented grid as v4, but the per-slot multiply is
split across engines: 208 seqs on DVE (2 chains, FD=416, PSUM->SBUF
tensor_mul) and 48 seqs on ScalarE-copy + GPSIMD-mul (2 chains, FD=96,
3-hop loop ~1.2us that now fits inside the fatter slot period).
"""

import numpy as np

B, T, K = 2048, 256, 48
NCORE = 8
PP = 2 * K
S = T // 2
R = 4
W = 4
E = (S + (R - 1) * W) // R    # 38
SEG0 = E
SEGN = E - W
NSEQ = B // NCORE             # 256
JA = 104                      # seqs per DVE chain
JB = 24                       # seqs per ACT+GPSIMD chain
FDA = JA * R                  # 416
FDB = JB * R                  # 96
NCOL = 2 * FDA + 2 * FDB      # 1024
CE = 5
NQ = E // CE
BIAS_C = -4.33
START, STOP = 46, 47

SEG_START = [0] + [SEG0 + SEGN * (k - 1) for k in range(1, R)]
assert SEG_START[-1] + SEGN == S
# chain table: (kind, seq_lo, J, col_offset)
CHAINS = [("A", 0, JA, 0), ("A", JA, JA, FDA),
          ("B", 2 * JA, JB, 2 * FDA), ("B", 2 * JA + JB, JB, 2 * FDA + FDB)]

_cache = {}


def _t_fwd():
    ti = np.empty((R, E), dtype=np.int64)
    for k in range(R):
        ti[k] = (np.arange(E) if k == 0
                 else SEG_START[k] - W + np.arange(E))
    return ti


def _build():
    import concourse.bass as bass
    import concourse.bacc as bacc
    import concourse.mybir as mybir
    from contextlib import ExitStack

    f32 = mybir.dt.float32
    bf16 = mybir.dt.bfloat16

    nc = bacc.Bacc(None, target_bir_lowering=False)

    fa = nc.dram_tensor("fa", [PP, E * NCOL], bf16, kind="ExternalInput")
    wmix = nc.dram_tensor("wmix", [PP, PP], bf16, kind="ExternalInput")
    init = nc.dram_tensor("init", [PP, FDA + FDB], bf16, kind="ExternalInput")
    xout = nc.dram_tensor("xout", [PP, 2 * NCOL], bf16, kind="ExternalOutput")

    with ExitStack() as ctx:
        sb = lambda shape, dt, name: ctx.enter_context(
            nc.sbuf_tensor(name, shape, dt))
        fa_sb = sb([PP, E * NCOL], bf16, "fa_sb")
        w_sb = sb([PP, PP], bf16, "w_sb")
        init_sb = sb([PP, FDA + FDB], bf16, "init_sb")
        xA = [sb([PP, FDA], bf16, f"xA{i}") for i in range(2)]
        uA = [sb([PP, FDA], bf16, f"uA{i}") for i in range(2)]
        xB = [sb([PP, FDB], bf16, f"xB{i}") for i in range(2)]
        uB = [sb([PP, FDB], bf16, f"uB{i}") for i in range(2)]
        yB = [[sb([PP, FDB], bf16, f"yB{i}_{p}") for p in range(2)]
              for i in range(2)]
        gdum = sb([PP, 8], bf16, "gdum")

        pA = [[nc.place_psum_tensor(f"pA{i}_{p}", [PP, FDA], f32,
                                    bank=i * 2 + p)
               for p in range(2)] for i in range(2)]
        pB = [[nc.place_psum_tensor(f"pB{i}_{p}", [PP, FDB], f32,
                                    bank=4 + i * 2 + p)
               for p in range(2)] for i in range(2)]

        sem = lambda name: ctx.enter_context(nc.semaphore(name))
        sAm = [sem(f"sA{i}m") for i in range(2)]
        sAt = [sem(f"sA{i}t") for i in range(2)]
        sBm = [sem(f"sB{i}m") for i in range(2)]
        sBc = [sem(f"sB{i}c") for i in range(2)]
        sBg = [sem(f"sB{i}g") for i in range(2)]
        dma_c = sem("dma_c")
        dma_fa = sem("dma_fa")
        dma_o = sem("dma_o")

        with nc.Block() as block:

            @block.sync
            def _(eng):
                eng.dma_start(w_sb[:], wmix[:]).then_inc(dma_c, 16)
                eng.dma_start(init_sb[:], init[:]).then_inc(dma_c, 16)
                qa = CE * NCOL
                for q in range(NQ):
                    eng.dma_start(fa_sb[:, q * qa:(q + 1) * qa],
                                  fa[:, q * qa:(q + 1) * qa]
                                  ).then_inc(dma_fa, 16)
                for i in range(2):
                    eng.wait_ge(sAt[i], E)
                    o = i * FDA
                    eng.dma_start(xout[:, o:o + FDA],
                                  xA[i][:]).then_inc(dma_o, 16)
                    eng.dma_start(xout[:, NCOL + o:NCOL + o + FDA],
                                  uA[i][:]).then_inc(dma_o, 16)
                for i in range(2):
                    eng.wait_ge(sBg[i], E)
                    o = 2 * FDA + i * FDB
                    eng.dma_start(xout[:, o:o + FDB],
                                  xB[i][:]).then_inc(dma_o, 16)
                    eng.dma_start(xout[:, NCOL + o:NCOL + o + FDB],
                                  uB[i][:]).then_inc(dma_o, 16)
                eng.wait_ge(dma_o, 128)

            @block.tensor
            def _(eng):
                eng.wait_ge(dma_c, 32)
                eng.ldweights(w_sb[:])
                for e in range(E):
                    par = e % 2
                    for i in range(2):   # B first: longest loop
                        rhs = (init_sb[:, FDA:FDA + FDB] if e == 0
                               else xB[i][:])
                        mm = nc.tensor.matmul(pB[i][par][:], w_sb[:], rhs,
                                              start=True, stop=True)
                        mm.ins.ldweights = False
                        if e > 0:
                            mm._wait_ge(sBg[i], e)
                        mm.then_inc(sBm[i], 1)
                    for i in range(2):
                        rhs = init_sb[:, 0:FDA] if e == 0 else xA[i][:]
                        mm = nc.tensor.matmul(pA[i][par][:], w_sb[:], rhs,
                                              start=True, stop=True)
                        mm.ins.ldweights = False
                        if e > 0:
                            mm._wait_ge(sAt[i], e)
                        mm.then_inc(sAm[i], 1)

            @block.vector
            def _(eng):
                for e in range(E):
                    par = e % 2
                    if e % CE == 0:
                        eng.wait_ge(dma_fa, 16 * (e // CE + 1))
                    for i in range(2):
                        off = e * NCOL + i * FDA
                        tt = nc.vector.tensor_mul(xA[i][:], pA[i][par][:],
                                                  fa_sb[:, off:off + FDA])
                        tt._wait_ge(sAm[i], e + 1)
                        tt.then_inc(sAt[i], 1)
                    if e == W - 1:
                        for i in range(2):
                            nc.vector.tensor_copy(uA[i][:], xA[i][:])

            @block.scalar
            def _(eng):
                for e in range(E):
                    par = e % 2
                    for i in range(2):
                        cp = nc.scalar.copy(yB[i][par][:], pB[i][par][:])
                        cp._wait_ge(sBm[i], e + 1)
                        cp.then_inc(sBc[i], 1)

            @block.gpsimd
            def _(eng):
                nc.gpsimd.memset(gdum[:], 0.0)
                nc.gpsimd.tensor_mul(gdum[:], gdum[:], gdum[:])
                for e in range(E):
                    par = e % 2
                    if e % CE == 0:
                        eng.wait_ge(dma_fa, 16 * (e // CE + 1))
                    for i in range(2):
                        off = e * NCOL + 2 * FDA + i * FDB
                        gt = nc.gpsimd.tensor_mul(xB[i][:], yB[i][par][:],
                                                  fa_sb[:, off:off + FDB])
                        gt._wait_ge(sBc[i], e + 1)
                        gt.then_inc(sBg[i], 1)
                    if e == W - 1:
                        for i in range(2):
                            nc.gpsimd.tensor_copy(uB[i][:], xB[i][:])

        nc.compile()
    return nc


def _col_perm():
    """Map (seq, k) -> physical column in [0, NCOL)."""
    P = np.empty((NSEQ, R), dtype=np.int64)
    for kind, lo, J, off in CHAINS:
        for j in range(J):
            for k in range(R):
                P[lo + j, k] = off + k * J + j
    return P


def _pack_host(feats, transitions):
    import ml_dtypes

    feats = np.asarray(feats, dtype=np.float32)
    trans = np.asarray(transitions, dtype=np.float64)

    TIf = _t_fwd()
    TIb = (T - 1) - TIf

    F = np.exp(feats).reshape(NCORE, NSEQ, T, K)
    fwd = F[:, :, TIf, :]              # [c, seq, R, E, K]
    bwd = F[:, :, TIb, :]
    pk = np.stack([fwd, bwd], axis=2)  # [c, seq, h, R, E, K]
    arr = pk.transpose(0, 2, 5, 4, 1, 3)   # [c, h, K, E, seq, R]
    arr = np.ascontiguousarray(arr).reshape(NCORE, PP, E, NSEQ * R)
    P = _col_perm().reshape(-1)        # src col seq*R+k -> dst P[...]
    out = np.empty_like(arr)
    out[..., P] = arr
    fa = out.reshape(NCORE, PP, E * NCOL).astype(ml_dtypes.bfloat16)

    Wm = np.exp(trans + BIAS_C)
    wmix = np.zeros((PP, PP), dtype=np.float64)
    wmix[:K, :K] = Wm.T
    wmix[K:, K:] = Wm
    wmix = wmix.astype(ml_dtypes.bfloat16)

    init = np.ones((PP, FDA + FDB), dtype=np.float64)
    for seg0_cols in (slice(0, JA), slice(FDA, FDA + JB)):
        init[:, seg0_cols] = 0.0
        init[START, seg0_cols] = 1.0
        init[K + STOP, seg0_cols] = 1.0
    init = init.astype(ml_dtypes.bfloat16)

    shared = {"wmix": wmix, "init": init}
    return fa, shared


def _postprocess(results, transitions):
    trans = np.asarray(transitions, dtype=np.float64)
    Wn = np.exp(trans)
    out = np.empty((NCORE, NSEQ), dtype=np.float64)
    for core in range(NCORE):
        xf = np.asarray(results[core]["xout"], dtype=np.float64)
        for kind, lo, J, off in CHAINS:
            FDc = J * R
            wfin = xf[:, off:off + FDc].reshape(PP, R, J)
            uu = xf[:, NCOL + off:NCOL + off + FDc].reshape(PP, R, J)
            Cs = np.zeros(J)
            for rows in (slice(0, K), slice(K, PP)):
                mw = wfin[rows].mean(axis=0)
                mu = uu[rows].mean(axis=0)
                for k in range(1, R):
                    Cs += np.log(mw[k - 1]) - np.log(mu[k])
            Ef = wfin[:K, R - 1, :]
            Gf = wfin[K:, R - 1, :]
            z = np.sum((Wn @ Ef) * Gf, axis=0)
            out[core, lo:lo + J] = np.log(z) + Cs - T * BIAS_C
    return out.reshape(B).astype(np.float32)


def _simulate(fa, shared):
    import ml_dtypes
    results = []
    Wmix = np.asarray(shared["wmix"], dtype=np.float64)
    init = np.asarray(shared["init"], dtype=np.float64)
    for core in range(NCORE):
        F = np.asarray(fa[core], dtype=np.float64).reshape(PP, E, NCOL)
        xo = np.zeros((PP, 2 * NCOL))
        for kind, lo, J, off in CHAINS:
            FDc = J * R
            ini = (init[:, 0:FDA] if kind == "A"
                   else init[:, FDA:FDA + FDB])
            x = ini[:, :FDc].copy()
            for e in range(E):
                x = (Wmix.T @ x) * F[:, e, off:off + FDc]
                x = x.astype(ml_dtypes.bfloat16).astype(np.float64)
                if e == W - 1:
                    xo[:, NCOL + off:NCOL + off + FDc] = x
            xo[:, off:off + FDc] = x
        results.append({"xout": xo.astype(ml_dtypes.bfloat16)})
    return results


def kernel(feats, transitions):
    from concourse.bass_utils import run_bass_kernel_spmd

    fa, shared = _pack_host(feats, transitions)
    if "nc" not in _cache:
        _cache["nc"] = _build()
    nc = _cache["nc"]

    in_maps = [dict(shared, fa=fa[c]) for c in range(NCORE)]
    res = run_bass_kernel_spmd(nc, in_maps, list(range(NCORE)))
    return _postprocess(res.results, transitions)


# revision 6
# speedup vs baseline: 1.1795x; 1.1795x over previous
"""Trainium2 raw-Bass CRF kernel, v5: v4 time-segmentation + second
elementwise path.

Same 38-slot R=4 segmented grid as v4, but the per-slot multiply is
split across engines: 208 seqs on DVE (2 chains, FD=416, PSUM->SBUF
tensor_mul) and 48 seqs on ScalarE-copy + GPSIMD-mul (2 chains, FD=96,
3-hop loop ~1.2us that now fits inside the fatter slot period).
"""

import numpy as np

B, T, K = 2048, 256, 48
NCORE = 8
PP = 2 * K
S = T // 2
R = 4
W = 4
E = (S + (R - 1) * W) // R    # 38
SEG0 = E
SEGN = E - W
NSEQ = B // NCORE             # 256
JA = 104                      # seqs per DVE chain
JB = 24                       # seqs per ACT+GPSIMD chain
FDA = JA * R                  # 416
FDB = JB * R                  # 96
NCOL = 2 * FDA + 2 * FDB      # 1024
# F-DMA chunk sizes in slots: small first chunks for a fast pipeline
# start, large later ones to keep consumer-side waits rare.
CHUNKS = [1, 1, 1, 2, 5, 5, 5, 5, 5, 5]
CHUNK_START = [sum(CHUNKS[:q]) for q in range(len(CHUNKS))]
NQ = len(CHUNKS)
BIAS_C = -4.33
START, STOP = 46, 47

SEG_START = [0] + [SEG0 + SEGN * (k - 1) for k in range(1, R)]
assert SEG_START[-1] + SEGN == S
# chain table: (kind, seq_lo, J, col_offset)
CHAINS = [("A", 0, JA, 0), ("A", JA, JA, FDA),
          ("B", 2 * JA, JB, 2 * FDA), ("B", 2 * JA + JB, JB, 2 * FDA + FDB)]

_cache = {}


def _t_fwd():
    ti = np.empty((R, E), dtype=np.int64)
    for k in range(R):
        ti[k] = (np.arange(E) if k == 0
                 else SEG_START[k] - W + np.arange(E))
    return ti


def _build():
    import concourse.bass as bass
    import concourse.bacc as bacc
    import concourse.mybir as mybir
    from contextlib import ExitStack

    f32 = mybir.dt.float32
    bf16 = mybir.dt.bfloat16

    nc = bacc.Bacc(None, target_bir_lowering=False)

    fa = nc.dram_tensor("fa", [PP, E * NCOL], bf16, kind="ExternalInput")
    wmix = nc.dram_tensor("wmix", [PP, PP], bf16, kind="ExternalInput")
    init = nc.dram_tensor("init", [PP, FDA + FDB], bf16, kind="ExternalInput")
    xout = nc.dram_tensor("xout", [PP, 2 * NCOL], bf16, kind="ExternalOutput")

    with ExitStack() as ctx:
        sb = lambda shape, dt, name: ctx.enter_context(
            nc.sbuf_tensor(name, shape, dt))
        fa_sb = sb([PP, E * NCOL], bf16, "fa_sb")
        w_sb = sb([PP, PP], bf16, "w_sb")
        init_sb = sb([PP, FDA + FDB], bf16, "init_sb")
        xA = [sb([PP, FDA], bf16, f"xA{i}") for i in range(2)]
        uA = [sb([PP, FDA], bf16, f"uA{i}") for i in range(2)]
        xB = [sb([PP, FDB], bf16, f"xB{i}") for i in range(2)]
        uB = [sb([PP, FDB], bf16, f"uB{i}") for i in range(2)]
        yB = [[sb([PP, FDB], bf16, f"yB{i}_{p}") for p in range(2)]
              for i in range(2)]
        gdum = sb([PP, 8], bf16, "gdum")

        pA = [[nc.place_psum_tensor(f"pA{i}_{p}", [PP, FDA], f32,
                                    bank=i * 2 + p)
               for p in range(2)] for i in range(2)]
        pB = [[nc.place_psum_tensor(f"pB{i}_{p}", [PP, FDB], f32,
                                    bank=4 + i * 2 + p)
               for p in range(2)] for i in range(2)]

        sem = lambda name: ctx.enter_context(nc.semaphore(name))
        sAm = [sem(f"sA{i}m") for i in range(2)]
        sAt = [sem(f"sA{i}t") for i in range(2)]
        sBm = [sem(f"sB{i}m") for i in range(2)]
        sBc = [sem(f"sB{i}c") for i in range(2)]
        sBg = [sem(f"sB{i}g") for i in range(2)]
        dma_c = sem("dma_c")
        dma_fa = sem("dma_fa")
        dma_o = sem("dma_o")

        with nc.Block() as block:

            @block.sync
            def _(eng):
                eng.dma_start(w_sb[:], wmix[:]).then_inc(dma_c, 16)
                eng.dma_start(init_sb[:], init[:]).then_inc(dma_c, 16)
                for q in range(NQ):
                    lo = CHUNK_START[q] * NCOL
                    hi = lo + CHUNKS[q] * NCOL
                    eng.dma_start(fa_sb[:, lo:hi],
                                  fa[:, lo:hi]).then_inc(dma_fa, 16)
                for i in range(2):
                    eng.wait_ge(sAt[i], E)
                    o = i * FDA
                    eng.dma_start(xout[:, o:o + FDA],
                                  xA[i][:]).then_inc(dma_o, 16)
                    eng.dma_start(xout[:, NCOL + o:NCOL + o + FDA],
                                  uA[i][:]).then_inc(dma_o, 16)
                for i in range(2):
                    eng.wait_ge(sBg[i], E)
                    o = 2 * FDA + i * FDB
                    eng.dma_start(xout[:, o:o + FDB],
                                  xB[i][:]).then_inc(dma_o, 16)
                    eng.dma_start(xout[:, NCOL + o:NCOL + o + FDB],
                                  uB[i][:]).then_inc(dma_o, 16)
                eng.wait_ge(dma_o, 128)

            @block.tensor
            def _(eng):
                eng.wait_ge(dma_c, 32)
                eng.ldweights(w_sb[:])
                for e in range(E):
                    par = e % 2
                    for i in range(2):   # B first: longest loop
                        rhs = (init_sb[:, FDA:FDA + FDB] if e == 0
                               else xB[i][:])
                        mm = nc.tensor.matmul(pB[i][par][:], w_sb[:], rhs,
                                              start=True, stop=True)
                        mm.ins.ldweights = False
                        if e > 0:
                            mm._wait_ge(sBg[i], e)
                        mm.then_inc(sBm[i], 1)
                    for i in range(2):
                        rhs = init_sb[:, 0:FDA] if e == 0 else xA[i][:]
                        mm = nc.tensor.matmul(pA[i][par][:], w_sb[:], rhs,
                                              start=True, stop=True)
                        mm.ins.ldweights = False
                        if e > 0:
                            mm._wait_ge(sAt[i], e)
                        mm.then_inc(sAm[i], 1)

            @block.vector
            def _(eng):
                for e in range(E):
                    par = e % 2
                    if e in CHUNK_START:
                        eng.wait_ge(dma_fa,
                                    16 * (CHUNK_START.index(e) + 1))
                    for i in range(2):
                        off = e * NCOL + i * FDA
                        tt = nc.vector.tensor_mul(xA[i][:], pA[i][par][:],
                                                  fa_sb[:, off:off + FDA])
                        tt._wait_ge(sAm[i], e + 1)
                        tt.then_inc(sAt[i], 1)
                    if e == W - 1:
                        for i in range(2):
                            nc.vector.tensor_copy(uA[i][:], xA[i][:])

            @block.scalar
            def _(eng):
                for e in range(E):
                    par = e % 2
                    for i in range(2):
                        cp = nc.scalar.copy(yB[i][par][:], pB[i][par][:])
                        cp._wait_ge(sBm[i], e + 1)
                        cp.then_inc(sBc[i], 1)

            @block.gpsimd
            def _(eng):
                nc.gpsimd.memset(gdum[:], 0.0)
                nc.gpsimd.tensor_mul(gdum[:], gdum[:], gdum[:])
                for e in range(E):
                    par = e % 2
                    if e in CHUNK_START:
                        eng.wait_ge(dma_fa,
                                    16 * (CHUNK_START.index(e) + 1))
                    for i in range(2):
                        off = e * NCOL + 2 * FDA + i * FDB
                        gt = nc.gpsimd.tensor_mul(xB[i][:], yB[i][par][:],
                                                  fa_sb[:, off:off + FDB])
                        gt._wait_ge(sBc[i], e + 1)
                        gt.then_inc(sBg[i], 1)
                    if e == W - 1:
                        for i in range(2):
                            nc.gpsimd.tensor_copy(uB[i][:], xB[i][:])

        nc.compile()
    return nc


def _col_perm():
    """Map (seq, k) -> physical column in [0, NCOL)."""
    P = np.empty((NSEQ, R), dtype=np.int64)
    for kind, lo, J, off in CHAINS:
        for j in range(J):
            for k in range(R):
                P[lo + j, k] = off + k * J + j
    return P


def _pack_host(feats, transitions):
    import ml_dtypes

    feats = np.asarray(feats, dtype=np.float32)
    trans = np.asarray(transitions, dtype=np.float64)

    TIf = _t_fwd()
    TIb = (T - 1) - TIf

    F = np.exp(feats).reshape(NCORE, NSEQ, T, K)
    fwd = F[:, :, TIf, :]              # [c, seq, R, E, K]
    bwd = F[:, :, TIb, :]
    pk = np.stack([fwd, bwd], axis=2)  # [c, seq, h, R, E, K]
    arr = pk.transpose(0, 2, 5, 4, 1, 3)   # [c, h, K, E, seq, R]
    arr = np.ascontiguousarray(arr).reshape(NCORE, PP, E, NSEQ * R)
    P = _col_perm().reshape(-1)        # src col seq*R+k -> dst P[...]
    out = np.empty_like(arr)
    out[..., P] = arr
    fa = out.reshape(NCORE, PP, E * NCOL).astype(ml_dtypes.bfloat16)

    Wm = np.exp(trans + BIAS_C)
    wmix = np.zeros((PP, PP), dtype=np.float64)
    wmix[:K, :K] = Wm.T
    wmix[K:, K:] = Wm
    wmix = wmix.astype(ml_dtypes.bfloat16)

    init = np.ones((PP, FDA + FDB), dtype=np.float64)
    for seg0_cols in (slice(0, JA), slice(FDA, FDA + JB)):
        init[:, seg0_cols] = 0.0
        init[START, seg0_cols] = 1.0
        init[K + STOP, seg0_cols] = 1.0
    init = init.astype(ml_dtypes.bfloat16)

    shared = {"wmix": wmix, "init": init}
    return fa, shared


def _postprocess(results, transitions):
    trans = np.asarray(transitions, dtype=np.float64)
    Wn = np.exp(trans)
    out = np.empty((NCORE, NSEQ), dtype=np.float64)
    for core in range(NCORE):
        xf = np.asarray(results[core]["xout"], dtype=np.float64)
        for kind, lo, J, off in CHAINS:
            FDc = J * R
            wfin = xf[:, off:off + FDc].reshape(PP, R, J)
            uu = xf[:, NCOL + off:NCOL + off + FDc].reshape(PP, R, J)
            Cs = np.zeros(J)
            for rows in (slice(0, K), slice(K, PP)):
                mw = wfin[rows].mean(axis=0)
                mu = uu[rows].mean(axis=0)
                for k in range(1, R):
                    Cs += np.log(mw[k - 1]) - np.log(mu[k])
            Ef = wfin[:K, R - 1, :]
            Gf = wfin[K:, R - 1, :]
            z = np.sum((Wn @ Ef) * Gf, axis=0)
            out[core, lo:lo + J] = np.log(z) + Cs - T * BIAS_C
    return out.reshape(B).astype(np.float32)


def _simulate(fa, shared):
    import ml_dtypes
    results = []
    Wmix = np.asarray(shared["wmix"], dtype=np.float64)
    init = np.asarray(shared["init"], dtype=np.float64)
    for core in range(NCORE):
        F = np.asarray(fa[core], dtype=np.float64).reshape(PP, E, NCOL)
        xo = np.zeros((PP, 2 * NCOL))
        for kind, lo, J, off in CHAINS:
            FDc = J * R
            ini = (init[:, 0:FDA] if kind == "A"
                   else init[:, FDA:FDA + FDB])
            x = ini[:, :FDc].copy()
            for e in range(E):
                x = (Wmix.T @ x) * F[:, e, off:off + FDc]
                x = x.astype(ml_dtypes.bfloat16).astype(np.float64)
                if e == W - 1:
                    xo[:, NCOL + off:NCOL + off + FDc] = x
            xo[:, off:off + FDc] = x
        results.append({"xout": xo.astype(ml_dtypes.bfloat16)})
    return results


def kernel(feats, transitions):
    from concourse.bass_utils import run_bass_kernel_spmd

    fa, shared = _pack_host(feats, transitions)
    if "nc" not in _cache:
        _cache["nc"] = _build()
    nc = _cache["nc"]

    in_maps = [dict(shared, fa=fa[c]) for c in range(NCORE)]
    res = run_bass_kernel_spmd(nc, in_maps, list(range(NCORE)))
    return _postprocess(res.results, transitions)


# revision 8
# speedup vs baseline: 1.2086x; 1.0247x over previous
"""Trainium2 raw-Bass CRF kernel, v5: v4 time-segmentation + second
elementwise path.

Same 38-slot R=4 segmented grid as v4, but the per-slot multiply is
split across engines: 208 seqs on DVE (2 chains, FD=416, PSUM->SBUF
tensor_mul) and 48 seqs on ScalarE-copy + GPSIMD-mul (2 chains, FD=96,
3-hop loop ~1.2us that now fits inside the fatter slot period).
"""

import numpy as np

B, T, K = 2048, 256, 48
NCORE = 8
PP = 2 * K
S = T // 2
R = 4
W = 4
E = (S + (R - 1) * W) // R    # 38
SEG0 = E
SEGN = E - W
NSEQ = B // NCORE             # 256
JA = 104                      # seqs per DVE chain
JB = 24                       # seqs per ACT+GPSIMD chain
FDA = JA * R                  # 416
FDB = JB * R                  # 96
NCOL = 2 * FDA + 2 * FDB      # 1024
# F-DMA chunk sizes in slots: small first chunks for a fast pipeline
# start, large later ones to keep consumer-side waits rare.
CHUNKS = [1, 1, 1, 2, 5, 5, 5, 5, 5, 5]
CHUNK_START = [sum(CHUNKS[:q]) for q in range(len(CHUNKS))]
NQ = len(CHUNKS)
BIAS_C = -4.33
START, STOP = 46, 47

SEG_START = [0] + [SEG0 + SEGN * (k - 1) for k in range(1, R)]
assert SEG_START[-1] + SEGN == S
# chain table: (kind, seq_lo, J, col_offset)
CHAINS = [("A", 0, JA, 0), ("A", JA, JA, FDA),
          ("B", 2 * JA, JB, 2 * FDA), ("B", 2 * JA + JB, JB, 2 * FDA + FDB)]

_cache = {}


def _t_fwd():
    ti = np.empty((R, E), dtype=np.int64)
    for k in range(R):
        ti[k] = (np.arange(E) if k == 0
                 else SEG_START[k] - W + np.arange(E))
    return ti


def _build():
    import concourse.bass as bass
    import concourse.bacc as bacc
    import concourse.mybir as mybir
    from contextlib import ExitStack

    f32 = mybir.dt.float32
    bf16 = mybir.dt.bfloat16

    nc = bacc.Bacc(None, target_bir_lowering=False)

    fa = nc.dram_tensor("fa", [PP, E * NCOL], bf16, kind="ExternalInput")
    consts = nc.dram_tensor("consts", [PP, PP + FDA + FDB], bf16,
                            kind="ExternalInput")
    xout = nc.dram_tensor("xout", [PP, 2 * NCOL], bf16, kind="ExternalOutput")

    with ExitStack() as ctx:
        sb = lambda shape, dt, name: ctx.enter_context(
            nc.sbuf_tensor(name, shape, dt))
        fa_sb = sb([PP, E * NCOL], bf16, "fa_sb")
        c_sb = sb([PP, PP + FDA + FDB], bf16, "c_sb")
        w_sb = c_sb[:, 0:PP]
        init_sb = c_sb[:, PP:]
        xA = [sb([PP, FDA], bf16, f"xA{i}") for i in range(2)]
        uA = [sb([PP, FDA], bf16, f"uA{i}") for i in range(2)]
        xB = [sb([PP, FDB], bf16, f"xB{i}") for i in range(2)]
        uB = [sb([PP, FDB], bf16, f"uB{i}") for i in range(2)]
        yB = [[sb([PP, FDB], bf16, f"yB{i}_{p}") for p in range(2)]
              for i in range(2)]
        gdum = sb([PP, 8], bf16, "gdum")

        pA = [[nc.place_psum_tensor(f"pA{i}_{p}", [PP, FDA], f32,
                                    bank=i * 2 + p)
               for p in range(2)] for i in range(2)]
        pB = [[nc.place_psum_tensor(f"pB{i}_{p}", [PP, FDB], f32,
                                    bank=4 + i * 2 + p)
               for p in range(2)] for i in range(2)]

        sem = lambda name: ctx.enter_context(nc.semaphore(name))
        sAm = [sem(f"sA{i}m") for i in range(2)]
        sAt = [sem(f"sA{i}t") for i in range(2)]
        sBm = [sem(f"sB{i}m") for i in range(2)]
        sBc = [sem(f"sB{i}c") for i in range(2)]
        sBg = [sem(f"sB{i}g") for i in range(2)]
        dma_c = sem("dma_c")
        dma_fa = sem("dma_fa")
        dma_o = sem("dma_o")

        with nc.Block() as block:

            @block.sync
            def _(eng):
                eng.dma_start(c_sb[:], consts[:]).then_inc(dma_c, 16)
                for q in range(NQ):
                    lo = CHUNK_START[q] * NCOL
                    hi = lo + CHUNKS[q] * NCOL
                    eng.dma_start(fa_sb[:, lo:hi],
                                  fa[:, lo:hi]).then_inc(dma_fa, 16)
                # u captures are ready after slot W+1: ship them mid-kernel
                for i in range(2):
                    eng.wait_ge(sAt[i], W + 2)
                    o = i * FDA
                    eng.dma_start(xout[:, NCOL + o:NCOL + o + FDA],
                                  uA[i][:]).then_inc(dma_o, 16)
                for i in range(2):
                    eng.wait_ge(sBg[i], W + 2)
                    o = 2 * FDA + i * FDB
                    eng.dma_start(xout[:, NCOL + o:NCOL + o + FDB],
                                  uB[i][:]).then_inc(dma_o, 16)
                for i in range(2):
                    eng.wait_ge(sBg[i], E)
                    o = 2 * FDA + i * FDB
                    eng.dma_start(xout[:, o:o + FDB],
                                  xB[i][:]).then_inc(dma_o, 16)
                eng.wait_ge(dma_o, 128)

            @block.tensor
            def _(eng):
                eng.wait_ge(dma_c, 16)
                eng.ldweights(w_sb)
                for e in range(E):
                    par = e % 2
                    for i in range(2):   # B first: longest loop
                        rhs = (init_sb[:, FDA:FDA + FDB] if e == 0
                               else xB[i][:])
                        mm = nc.tensor.matmul(pB[i][par][:], w_sb, rhs,
                                              start=True, stop=True)
                        mm.ins.ldweights = False
                        if e > 0:
                            mm._wait_ge(sBg[i], e)
                        mm.then_inc(sBm[i], 1)
                    for i in range(2):
                        rhs = init_sb[:, 0:FDA] if e == 0 else xA[i][:]
                        mm = nc.tensor.matmul(pA[i][par][:], w_sb, rhs,
                                              start=True, stop=True)
                        mm.ins.ldweights = False
                        if e > 0:
                            mm._wait_ge(sAt[i], e)
                        mm.then_inc(sAm[i], 1)

            @block.vector
            def _(eng):
                for e in range(E):
                    par = e % 2
                    if e in CHUNK_START:
                        eng.wait_ge(dma_fa,
                                    16 * (CHUNK_START.index(e) + 1))
                    for i in range(2):
                        off = e * NCOL + i * FDA
                        tt = nc.vector.tensor_mul(xA[i][:], pA[i][par][:],
                                                  fa_sb[:, off:off + FDA])
                        tt._wait_ge(sAm[i], e + 1)
                        tt.then_inc(sAt[i], 1)
                    if e == W - 1:
                        for i in range(2):
                            nc.vector.tensor_copy(uA[i][:], xA[i][:])

            @block.scalar
            def _(eng):
                for e in range(E):
                    par = e % 2
                    for i in range(2):
                        cp = nc.scalar.copy(yB[i][par][:], pB[i][par][:])
                        cp._wait_ge(sBm[i], e + 1)
                        cp.then_inc(sBc[i], 1)
                for i in range(2):
                    eng.wait_ge(sAt[i], E)
                    o = i * FDA
                    eng.dma_start(xout[:, o:o + FDA],
                                  xA[i][:]).then_inc(dma_o, 16)

            @block.gpsimd
            def _(eng):
                nc.gpsimd.memset(gdum[:], 0.0)
                nc.gpsimd.tensor_mul(gdum[:], gdum[:], gdum[:])
                for e in range(E):
                    par = e % 2
                    if e in CHUNK_START:
                        eng.wait_ge(dma_fa,
                                    16 * (CHUNK_START.index(e) + 1))
                    for i in range(2):
                        off = e * NCOL + 2 * FDA + i * FDB
                        gt = nc.gpsimd.tensor_mul(xB[i][:], yB[i][par][:],
                                                  fa_sb[:, off:off + FDB])
                        gt._wait_ge(sBc[i], e + 1)
                        gt.then_inc(sBg[i], 1)
                    if e == W - 1:
                        for i in range(2):
                            nc.gpsimd.tensor_copy(uB[i][:], xB[i][:])

        nc.compile()
    return nc


def _col_perm():
    """Map (seq, k) -> physical column in [0, NCOL)."""
    P = np.empty((NSEQ, R), dtype=np.int64)
    for kind, lo, J, off in CHAINS:
        for j in range(J):
            for k in range(R):
                P[lo + j, k] = off + k * J + j
    return P


def _pack_host(feats, transitions):
    import ml_dtypes

    feats = np.asarray(feats, dtype=np.float32)
    trans = np.asarray(transitions, dtype=np.float64)

    TIf = _t_fwd()
    TIb = (T - 1) - TIf

    F = np.exp(feats).reshape(NCORE, NSEQ, T, K)
    fwd = F[:, :, TIf, :]              # [c, seq, R, E, K]
    bwd = F[:, :, TIb, :]
    pk = np.stack([fwd, bwd], axis=2)  # [c, seq, h, R, E, K]
    arr = pk.transpose(0, 2, 5, 4, 1, 3)   # [c, h, K, E, seq, R]
    arr = np.ascontiguousarray(arr).reshape(NCORE, PP, E, NSEQ * R)
    P = _col_perm().reshape(-1)        # src col seq*R+k -> dst P[...]
    out = np.empty_like(arr)
    out[..., P] = arr
    fa = out.reshape(NCORE, PP, E * NCOL).astype(ml_dtypes.bfloat16)

    Wm = np.exp(trans + BIAS_C)
    wmix = np.zeros((PP, PP), dtype=np.float64)
    wmix[:K, :K] = Wm.T
    wmix[K:, K:] = Wm
    wmix = wmix.astype(ml_dtypes.bfloat16)

    init = np.ones((PP, FDA + FDB), dtype=np.float64)
    for seg0_cols in (slice(0, JA), slice(FDA, FDA + JB)):
        init[:, seg0_cols] = 0.0
        init[START, seg0_cols] = 1.0
        init[K + STOP, seg0_cols] = 1.0
    init = init.astype(ml_dtypes.bfloat16)

    shared = {"consts": np.concatenate([np.asarray(wmix), np.asarray(init)],
                                        axis=1)}
    return fa, shared


def _postprocess(results, transitions):
    trans = np.asarray(transitions, dtype=np.float64)
    Wn = np.exp(trans)
    out = np.empty((NCORE, NSEQ), dtype=np.float64)
    for core in range(NCORE):
        xf = np.asarray(results[core]["xout"], dtype=np.float64)
        for kind, lo, J, off in CHAINS:
            FDc = J * R
            wfin = xf[:, off:off + FDc].reshape(PP, R, J)
            uu = xf[:, NCOL + off:NCOL + off + FDc].reshape(PP, R, J)
            Cs = np.zeros(J)
            for rows in (slice(0, K), slice(K, PP)):
                mw = wfin[rows].mean(axis=0)
                mu = uu[rows].mean(axis=0)
                for k in range(1, R):
                    Cs += np.log(mw[k - 1]) - np.log(mu[k])
            Ef = wfin[:K, R - 1, :]
            Gf = wfin[K:, R - 1, :]
            z = np.sum((Wn @ Ef) * Gf, axis=0)
            out[core, lo:lo + J] = np.log(z) + Cs - T * BIAS_C
    return out.reshape(B).astype(np.float32)


def _simulate(fa, shared):
    import ml_dtypes
    results = []
    Wmix = np.asarray(shared["wmix"], dtype=np.float64)
    init = np.asarray(shared["init"], dtype=np.float64)
    for core in range(NCORE):
        F = np.asarray(fa[core], dtype=np.float64).reshape(PP, E, NCOL)
        xo = np.zeros((PP, 2 * NCOL))
        for kind, lo, J, off in CHAINS:
            FDc = J * R
            ini = (init[:, 0:FDA] if kind == "A"
                   else init[:, FDA:FDA + FDB])
            x = ini[:, :FDc].copy()
            for e in range(E):
                x = (Wmix.T @ x) * F[:, e, off:off + FDc]
                x = x.astype(ml_dtypes.bfloat16).astype(np.float64)
                if e == W - 1:
                    xo[:, NCOL + off:NCOL + off + FDc] = x
            xo[:, off:off + FDc] = x
        results.append({"xout": xo.astype(ml_dtypes.bfloat16)})
    return results


def kernel(feats, transitions):
    from concourse.bass_utils import run_bass_kernel_spmd

    fa, shared = _pack_host(feats, transitions)
    if "nc" not in _cache:
        _cache["nc"] = _build()
    nc = _cache["nc"]

    in_maps = [dict(shared, fa=fa[c]) for c in range(NCORE)]
    res = run_bass_kernel_spmd(nc, in_maps, list(range(NCORE)))
    return _postprocess(res.results, transitions)
